# revision 9
# baseline (speedup 1.0000x reference)
"""Causal self-attention on 8 Trainium2 NeuronCores.

Full inputs in, full output out. Sharding: core c -> (batch b = c//2,
head-group hg = c%2 covering 8 of 16 heads). Each core computes QKV
projections for its head slice, causal flash-attention in a transposed
layout (S^T = keys x queries, so softmax denominators come from a ones
column appended to V and no on-device transposes are needed), and a
partial output projection over its 512 feature columns. The host sums
the two partials per batch and adds the bias.

v2 performance structure:
 - The attention kk-loop is software-pipelined with lookahead 3: the
   in-order PE queue is [S(0..2)] then {PV(k), S(k+3)}, so the scalar
   engine's Exp latency (~0.9us/block) is hidden behind two full
   rounds of PE work and the PE never idles (idling drops its DVFS
   clock from ~2.4GHz to ~1.7GHz, which is what made the baseline's
   PV matmuls 40% slow).
 - PSUM: st triple-buffered (6 banks) + pv accumulators (2 banks) = 8.
   Output-projection tiles rotate through the st pool slots.
 - The softmax denominator broadcast is a gpsimd partition_broadcast
   instead of a PE matmul (saves 12.8us of PE).
 - Projection matmuls for query-block qb are injected one-per-round
   into qb+1's first attention stream so the PE never drains at block
   boundaries.
 - PE warm-up matmuls read a gpsimd-memset tile, not a DMA'd identity,
   so the clock ramps while input DMAs are still in flight.
 - Phase-1 PSUM evacuations alternate vector/scalar (scalar is
   otherwise idle until attention starts).
"""
import sys

if "/opt/trn_rl_repo" not in sys.path:
    sys.path.insert(0, "/opt/trn_rl_repo")

import numpy as np

import concourse.bass as bass
import concourse.tile as tile
from concourse import bacc, mybir
from concourse.bass_utils import run_bass_kernel_spmd

F32 = mybir.dt.float32
F16 = mybir.dt.float16
AF = mybir.ActivationFunctionType

B, T, C = 4, 2048, 1024
H, D = 16, 64
N_CORES = 8
HPC = 8            # heads per core
FPC = HPC * D      # feats per core = 512
QB = 512           # query block
NQB = T // QB      # 4
NKK = T // 128     # 16 key chunks
NCC = C // 128     # 8 contraction chunks
NFB = FPC // 128   # 4 feature blocks (head pairs)

_cached = {}


def _build_program():
    nc = bacc.Bacc("TRN2", target_bir_lowering=False, debug=False,
                   num_devices=N_CORES)

    xT_d = nc.dram_tensor("xT", [C, T], F16, kind="ExternalInput").ap()
    wqT_d = nc.dram_tensor("wqT", [C, FPC], F16, kind="ExternalInput").ap()
    wkT_d = nc.dram_tensor("wkT", [C, FPC], F16, kind="ExternalInput").ap()
    wvT_d = nc.dram_tensor("wvT", [C, FPC], F16, kind="ExternalInput").ap()
    wpT_d = nc.dram_tensor("wpT", [FPC, C], F16, kind="ExternalInput").ap()
    tri_d = nc.dram_tensor("tri", [128, 128], F16, kind="ExternalInput").ap()
    out_d = nc.dram_tensor("out", [T, C], F32, kind="ExternalOutput").ap()

    with tile.TileContext(nc) as tc:
        with tc.tile_pool(name="persist", bufs=1) as persist:
            qt_sb = persist.tile([128, NFB, T], F16, tag="qt")
            kt_sb = persist.tile([128, NFB, T], F16, tag="kt")
            v_sb = persist.tile([128, NKK, HPC, D + 1], F16, tag="v")
            tri = persist.tile([128, 128], F16, tag="tri")
            warm_sb = persist.tile([128, 128], F16, tag="warm")
            wp_sb = persist.tile([128, NFB, C], F16, tag="wp")

            # warm-up input needs no DMA: memset on vector (gpsimd's first
            # instruction lands ~6us late; vector starts immediately)
            nc.vector.memset(warm_sb, 0.0)
            # ones column of V' (PV matmul then emits softmax denominators)
            nc.gpsimd.memset(v_sb[:, :, :, D:D + 1], 1.0)

            # ---------------- phase 1: QKV projections ----------------
            with tc.tile_pool(name="wqkv", bufs=1) as wqkv, \
                 tc.tile_pool(name="xt", bufs=2) as xtp, \
                 tc.tile_pool(name="ps1", bufs=4, space="PSUM") as ps1:
                wq_sb = wqkv.tile([128, NCC, FPC], F16, tag="wq")
                wk_sb = wqkv.tile([128, NCC, FPC], F16, tag="wk")
                wv_sb = wqkv.tile([128, NCC, FPC], F16, tag="wv")
                wq_r = wqT_d.rearrange("(c p) f -> p c f", p=128)
                wk_r = wkT_d.rearrange("(c p) f -> p c f", p=128)
                wv_r = wvT_d.rearrange("(c p) f -> p c f", p=128)
                xT_r = xT_d.rearrange("(c p) t -> p c t", p=128)

                # first token block's x and the Q weights land first so the
                # first chains can start early
                xt0 = xtp.tile([128, NCC, QB], F16, tag="xt", name="xt0")
                for cc in range(NCC):
                    nc.sync.dma_start(xt0[:, cc, :], xT_r[:, cc, 0:QB])
                for cc in range(NCC):
                    nc.sync.dma_start(wq_sb[:, cc, :], wq_r[:, cc, :])
                for cc in range(NCC):
                    nc.sync.dma_start(wk_sb[:, cc, :], wk_r[:, cc, :])
                for cc in range(NCC):
                    nc.sync.dma_start(wv_sb[:, cc, :], wv_r[:, cc, :])
                nc.sync.dma_start(tri, tri_d)
                nc.sync.dma_start(
                    wp_sb, wpT_d.rearrange("(c p) f -> p c f", p=128))

                # warm the PE clock while the input DMAs stream in
                warm = ps1.tile([128, 512], F32, tag="ps1", name="warm")
                for i in range(90):
                    nc.tensor.matmul(warm[:, 0:128], warm_sb, warm_sb,
                                     start=True, stop=True,
                                     skip_group_check=True)

                evac_i = [0]

                def evac(out_ap, in_ap):
                    if evac_i[0] % 2 == 0:
                        nc.vector.tensor_copy(out_ap, in_ap)
                    else:
                        nc.scalar.copy(out_ap, in_ap)
                    evac_i[0] += 1

                for tb in range(NQB):
                    if tb == 0:
                        xt = xt0
                    else:
                        xt = xtp.tile([128, NCC, QB], F16, tag="xt",
                                      name="xt")
                        for cc in range(NCC):
                            nc.sync.dma_start(
                                xt[:, cc, :],
                                xT_r[:, cc, tb * QB:(tb + 1) * QB])
                    for fb in range(NFB):
                        qps = ps1.tile([128, QB], F32, tag="ps1", name="qps")
                        for cc in range(NCC):
                            nc.tensor.matmul(
                                qps, wq_sb[:, cc, fb * 128:(fb + 1) * 128],
                                xt[:, cc, :],
                                start=(cc == 0), stop=(cc == NCC - 1))
                        evac(qt_sb[:, fb, tb * QB:(tb + 1) * QB], qps)
                        kps = ps1.tile([128, QB], F32, tag="ps1", name="kps")
                        for cc in range(NCC):
                            nc.tensor.matmul(
                                kps, wk_sb[:, cc, fb * 128:(fb + 1) * 128],
                                xt[:, cc, :],
                                start=(cc == 0), stop=(cc == NCC - 1))
                        evac(kt_sb[:, fb, tb * QB:(tb + 1) * QB], kps)
                    for tt in range(4):
                        vps = ps1.tile([128, FPC], F32, tag="ps1", name="vps")
                        for cc in range(NCC):
                            nc.tensor.matmul(
                                vps, xt[:, cc, tt * 128:(tt + 1) * 128],
                                wv_sb[:, cc, :],
                                start=(cc == 0), stop=(cc == NCC - 1))
                        evac(v_sb[:, tb * 4 + tt, :, 0:D],
                             vps.rearrange("p (h d) -> p h d", h=HPC))

            # ------------- phase 2: attention + projection -------------
            with tc.tile_pool(name="pt", bufs=4) as ptp, \
                 tc.tile_pool(name="yt", bufs=2) as ytp, \
                 tc.tile_pool(name="ytr", bufs=2) as ytrp, \
                 tc.tile_pool(name="sums", bufs=2) as sumsp, \
                 tc.tile_pool(name="bc", bufs=2) as bcp, \
                 tc.tile_pool(name="rcp", bufs=2) as rcpp, \
                 tc.tile_pool(name="outsb", bufs=3) as outp, \
                 tc.tile_pool(name="st", bufs=3, space="PSUM") as stp, \
                 tc.tile_pool(name="pv", bufs=1, space="PSUM") as pvp:

                tri_b = bass.AP(tri.tensor, tri.offset,
                                [tri.ap[0], [0, 2], tri.ap[1]])

                def attn_fb(qb, fb, yt, inject):
                    """One (query-block, feature-block) attention stream."""
                    nkk = 4 * qb + 4
                    pv = [pvp.tile([65, QB], F32, tag=f"pv{h2}",
                                   name=f"pv{h2}")
                          for h2 in range(2)]
                    pts = {}

                    def issue_S(k):
                        dl = k - 4 * qb
                        j0 = 128 * dl if dl >= 0 else 0
                        st = stp.tile([128, 2, QB], F32, tag="st", name="st")
                        for h2 in range(2):
                            p0, p1 = 64 * h2, 64 * h2 + 64
                            nc.tensor.matmul(
                                st[:, h2, j0:QB],
                                kt_sb[p0:p1, fb, k * 128:(k + 1) * 128],
                                qt_sb[p0:p1, fb, qb * QB + j0:(qb + 1) * QB],
                                start=True, stop=True,
                                skip_group_check=True)
                        pt = ptp.tile([128, 2, QB], F16, tag="pt", name="pt")
                        nc.scalar.activation(
                            pt[:, :, j0:QB], st[:, :, j0:QB], AF.Exp)
                        if dl >= 0:
                            # zero the causally-dead triangle of the diagonal
                            # band (both heads in one strided op; the 0-step
                            # middle dim re-reads the same mask tile)
                            band = pt[:, :, j0:j0 + 128]
                            nc.vector.tensor_mul(band, band, tri_b)
                        pts[k] = (pt, j0)

                    def issue_PV(k):
                        pt, j0 = pts.pop(k)
                        for h2 in range(2):
                            h = 2 * fb + h2
                            nc.tensor.matmul(
                                pv[h2][:, j0:QB], v_sb[:, k, h, :],
                                pt[:, h2, j0:QB],
                                start=(k == 0), stop=(k == nkk - 1),
                                skip_group_check=True)

                    LA = 3
                    for k in range(min(LA, nkk)):
                        issue_S(k)
                    for k in range(nkk):
                        issue_PV(k)
                        if k + LA < nkk:
                            issue_S(k + LA)
                        if inject and k >= 1:
                            inject.pop(0)()
                    while inject:
                        inject.pop(0)()

                    # epilogue fast part: one f32 copy frees each pv bank;
                    # the denominator row broadcasts across partitions via a
                    # stride-0 DMA (gpsimd's partition_broadcast has ~7us
                    # first-use latency; DMA queues are warm)
                    ytr = ytrp.tile([65, 2, QB], F32, tag="ytr", name="ytr")
                    bc = bcp.tile([64, 2, QB], F32, tag="bc", name="bc")
                    rcp = rcpp.tile([64, 2, QB], F32, tag="rcp", name="rcp")
                    for h2 in range(2):
                        nc.vector.tensor_copy(ytr[:, h2, :], pv[h2])
                        row = ytr[D:D + 1, h2, :]
                        row_b = bass.AP(row.tensor, row.offset,
                                        [row.ap[0], [0, 64], row.ap[-1]])
                        nc.sync.dma_start(bc[:, h2, :], row_b)

                    def finish():
                        # deferred: runs injected into the next stream, after
                        # the broadcast DMA has landed
                        for h2 in range(2):
                            nc.vector.reciprocal_approx_fast(
                                out=rcp[:, h2, :], in_=bc[:, h2, :])
                            nc.vector.tensor_mul(
                                yt[64 * h2:64 * h2 + 64, fb, :],
                                ytr[0:D, h2, :], rcp[:, h2, :])
                    return finish

                def make_prj(qb, yt, osbs):
                    """Closures for qb's 8 output-projection chains."""
                    def chain(j):
                        def run():
                            tt, ofc = j // 2, j % 2
                            if ofc == 0:
                                osbs.append(outp.tile([128, C], F32,
                                                      tag="osb", name="osb"))
                            osb = osbs[-1]
                            prjt = stp.tile([128, QB], F32, tag="st",
                                            name="prjt")
                            for cc in range(NFB):
                                nc.tensor.matmul(
                                    prjt,
                                    yt[:, cc, tt * 128:(tt + 1) * 128],
                                    wp_sb[:, cc, ofc * 512:(ofc + 1) * 512],
                                    start=(cc == 0), stop=(cc == NFB - 1),
                                    skip_group_check=True)
                            nc.vector.tensor_copy(
                                osb[:, ofc * 512:(ofc + 1) * 512], prjt)
                            if ofc == 1:
                                r0 = qb * QB + tt * 128
                                nc.sync.dma_start(out_d[r0:r0 + 128, :], osb)
                        return run
                    return [chain(j) for j in range(8)]

                yts = {}
                osbs = []
                pending_finish = None
                for qb in range(NQB):
                    yts[qb] = ytp.tile([128, NFB, QB], F16, tag="yt",
                                       name="yt")
                    for fb in range(NFB):
                        inject = []
                        if pending_finish is not None:
                            inject.append(pending_finish)
                        if fb == 0 and qb > 0:
                            inject.extend(make_prj(qb - 1, yts[qb - 1],
                                                   osbs))
                        pending_finish = attn_fb(qb, fb, yts[qb], inject)
                # drain: last stream's softmax finish + its projection
                pending_finish()
                for run in make_prj(NQB - 1, yts[NQB - 1], osbs):
                    run()

    nc.compile()
    return nc


def _host_inputs(x, Wk, Wq, Wv, Wp):
    """Build the 8 per-core input maps (host-side slicing/transposes)."""
    p = np.arange(128)[:, None]
    jj = np.arange(128)[None, :]
    tri_np = np.where(jj < p, 0.0, 1.0).astype(np.float16)

    in_maps = []
    for c in range(N_CORES):
        b, hg = c // 2, c % 2
        fs = slice(hg * FPC, (hg + 1) * FPC)
        in_maps.append({
            "xT": np.ascontiguousarray(x[b].T).astype(np.float16),
            "wqT": np.ascontiguousarray((Wq[fs, :] * 0.125).T).astype(np.float16),
            "wkT": np.ascontiguousarray(Wk[fs, :].T).astype(np.float16),
            "wvT": np.ascontiguousarray(Wv[fs, :].T).astype(np.float16),
            "wpT": np.ascontiguousarray(Wp[:, fs].T).astype(np.float16),
            "tri": tri_np,
        })
    return in_maps


def kernel(x, Wk, Wq, Wv, Wp, bp, _trace=False):
    x = np.asarray(x, dtype=np.float32)
    Wk = np.asarray(Wk, dtype=np.float32)
    Wq = np.asarray(Wq, dtype=np.float32)
    Wv = np.asarray(Wv, dtype=np.float32)
    Wp = np.asarray(Wp, dtype=np.float32)
    bp = np.asarray(bp, dtype=np.float32)

    if "nc" not in _cached:
        _cached["nc"] = _build_program()
    nc = _cached["nc"]

    in_maps = _host_inputs(x, Wk, Wq, Wv, Wp)
    res = run_bass_kernel_spmd(nc, in_maps, core_ids=list(range(N_CORES)),
                               trace=_trace)
    _cached["last_result"] = res

    out = np.empty((B, T, C), dtype=np.float32)
    for b in range(B):
        out[b] = (res.results[2 * b]["out"].astype(np.float32)
                  + res.results[2 * b + 1]["out"]
                  + bp[None, :])
    return out


# revision 11
# speedup vs baseline: 1.3121x; 1.3121x over previous
"""Causal self-attention on 8 Trainium2 NeuronCores.

Full inputs in, full output out. Sharding: core c -> (batch b = c//2,
head-group hg = c%2 covering 8 of 16 heads). Each core computes QKV
projections for its head slice, causal flash-attention in a transposed
layout (S^T = keys x queries, so softmax denominators come from a ones
column appended to V and no on-device transposes are needed), and a
partial output projection over its 512 feature columns. The host sums
the two partials per batch and adds the bias.

v2 performance structure:
 - The attention kk-loop is software-pipelined with lookahead 3: the
   in-order PE queue is [S(0..2)] then {PV(k), S(k+3)}, so the scalar
   engine's Exp latency (~0.9us/block) is hidden behind two full
   rounds of PE work and the PE never idles (idling drops its DVFS
   clock from ~2.4GHz to ~1.7GHz, which is what made the baseline's
   PV matmuls 40% slow).
 - PSUM: st triple-buffered (6 banks) + pv accumulators (2 banks) = 8.
   Output-projection tiles rotate through the st pool slots.
 - The softmax denominator broadcast is a gpsimd partition_broadcast
   instead of a PE matmul (saves 12.8us of PE).
 - Projection matmuls for query-block qb are injected one-per-round
   into qb+1's first attention stream so the PE never drains at block
   boundaries.
 - PE warm-up matmuls read a gpsimd-memset tile, not a DMA'd identity,
   so the clock ramps while input DMAs are still in flight.
 - Phase-1 PSUM evacuations alternate vector/scalar (scalar is
   otherwise idle until attention starts).
"""
import sys

if "/opt/trn_rl_repo" not in sys.path:
    sys.path.insert(0, "/opt/trn_rl_repo")

import numpy as np

import concourse.bass as bass
import concourse.tile as tile
from concourse import bacc, mybir
from concourse.bass_utils import run_bass_kernel_spmd

F32 = mybir.dt.float32
F16 = mybir.dt.float16
AF = mybir.ActivationFunctionType

B, T, C = 4, 2048, 1024
H, D = 16, 64
N_CORES = 8
HPC = 8            # heads per core
FPC = HPC * D      # feats per core = 512
QB = 512           # query block
NQB = T // QB      # 4
NKK = T // 128     # 16 key chunks
NCC = C // 128     # 8 contraction chunks
NFB = FPC // 128   # 4 feature blocks (head pairs)

_cached = {}


def _build_program():
    nc = bacc.Bacc("TRN2", target_bir_lowering=False, debug=False,
                   num_devices=N_CORES)

    xT_d = nc.dram_tensor("xT", [C, T], F16, kind="ExternalInput").ap()
    wqT_d = nc.dram_tensor("wqT", [C, FPC], F16, kind="ExternalInput").ap()
    wkT_d = nc.dram_tensor("wkT", [C, FPC], F16, kind="ExternalInput").ap()
    wvT_d = nc.dram_tensor("wvT", [C, FPC], F16, kind="ExternalInput").ap()
    wpT_d = nc.dram_tensor("wpT", [FPC, C], F16, kind="ExternalInput").ap()
    tri_d = nc.dram_tensor("tri", [128, 128], F16, kind="ExternalInput").ap()
    out_d = nc.dram_tensor("out", [T, C], F32, kind="ExternalOutput").ap()

    with tile.TileContext(nc) as tc:
        with tc.tile_pool(name="persist", bufs=1) as persist:
            qt_sb = persist.tile([128, NFB, T], F16, tag="qt")
            kt_sb = persist.tile([128, NFB, T], F16, tag="kt")
            v_sb = persist.tile([128, NKK, HPC, D + 1], F16, tag="v")
            tri = persist.tile([128, 128], F16, tag="tri")
            warm_sb = persist.tile([128, 128], F16, tag="warm")
            wp_sb = persist.tile([128, NFB, C], F16, tag="wp")
            ones_col = persist.tile([1, 64], F16, tag="ones")

            # warm-up input needs no DMA: memset on vector (gpsimd's first
            # instruction lands ~6us late; vector starts immediately)
            nc.vector.memset(warm_sb, 0.0)
            nc.vector.memset(ones_col, 1.0)
            # ones column of V' (PV matmul then emits softmax denominators)
            nc.gpsimd.memset(v_sb[:, :, :, D:D + 1], 1.0)

            # ---------------- phase 1: QKV projections ----------------
            with tc.tile_pool(name="wqkv", bufs=1) as wqkv, \
                 tc.tile_pool(name="xt", bufs=2) as xtp, \
                 tc.tile_pool(name="ps1", bufs=4, space="PSUM") as ps1:
                wq_sb = wqkv.tile([128, NCC, FPC], F16, tag="wq")
                wk_sb = wqkv.tile([128, NCC, FPC], F16, tag="wk")
                wv_sb = wqkv.tile([128, NCC, FPC], F16, tag="wv")
                wq_r = wqT_d.rearrange("(c p) f -> p c f", p=128)
                wk_r = wkT_d.rearrange("(c p) f -> p c f", p=128)
                wv_r = wvT_d.rearrange("(c p) f -> p c f", p=128)
                xT_r = xT_d.rearrange("(c p) t -> p c t", p=128)

                # first token block's x and the Q weights land first so the
                # first chains can start early
                xt0 = xtp.tile([128, NCC, QB], F16, tag="xt", name="xt0")
                for cc in range(NCC):
                    nc.sync.dma_start(xt0[:, cc, :], xT_r[:, cc, 0:QB])
                for cc in range(NCC):
                    nc.sync.dma_start(wq_sb[:, cc, :], wq_r[:, cc, :])
                for cc in range(NCC):
                    nc.sync.dma_start(wk_sb[:, cc, :], wk_r[:, cc, :])
                for cc in range(NCC):
                    nc.sync.dma_start(wv_sb[:, cc, :], wv_r[:, cc, :])
                nc.sync.dma_start(tri, tri_d)
                nc.sync.dma_start(
                    wp_sb, wpT_d.rearrange("(c p) f -> p c f", p=128))

                # warm the PE clock while the input DMAs stream in
                warm = ps1.tile([128, 512], F32, tag="ps1", name="warm")
                for i in range(90):
                    nc.tensor.matmul(warm[:, 0:128], warm_sb, warm_sb,
                                     start=True, stop=True,
                                     skip_group_check=True)

                evac_i = [0]

                def evac(out_ap, in_ap):
                    if evac_i[0] % 2 == 0:
                        nc.vector.tensor_copy(out_ap, in_ap)
                    else:
                        nc.scalar.copy(out_ap, in_ap)
                    evac_i[0] += 1

                for tb in range(NQB):
                    if tb == 0:
                        xt = xt0
                    else:
                        xt = xtp.tile([128, NCC, QB], F16, tag="xt",
                                      name="xt")
                        for cc in range(NCC):
                            nc.sync.dma_start(
                                xt[:, cc, :],
                                xT_r[:, cc, tb * QB:(tb + 1) * QB])
                    for fb in range(NFB):
                        qps = ps1.tile([128, QB], F32, tag="ps1", name="qps")
                        for cc in range(NCC):
                            nc.tensor.matmul(
                                qps, wq_sb[:, cc, fb * 128:(fb + 1) * 128],
                                xt[:, cc, :],
                                start=(cc == 0), stop=(cc == NCC - 1))
                        evac(qt_sb[:, fb, tb * QB:(tb + 1) * QB], qps)
                        kps = ps1.tile([128, QB], F32, tag="ps1", name="kps")
                        for cc in range(NCC):
                            nc.tensor.matmul(
                                kps, wk_sb[:, cc, fb * 128:(fb + 1) * 128],
                                xt[:, cc, :],
                                start=(cc == 0), stop=(cc == NCC - 1))
                        evac(kt_sb[:, fb, tb * QB:(tb + 1) * QB], kps)
                    for tt in range(4):
                        vps = ps1.tile([128, FPC], F32, tag="ps1", name="vps")
                        for cc in range(NCC):
                            nc.tensor.matmul(
                                vps, xt[:, cc, tt * 128:(tt + 1) * 128],
                                wv_sb[:, cc, :],
                                start=(cc == 0), stop=(cc == NCC - 1))
                        evac(v_sb[:, tb * 4 + tt, :, 0:D],
                             vps.rearrange("p (h d) -> p h d", h=HPC))

            # ------------- phase 2: attention + projection -------------
            with tc.tile_pool(name="pt", bufs=4) as ptp, \
                 tc.tile_pool(name="yt", bufs=2) as ytp, \
                 tc.tile_pool(name="ytr", bufs=2) as ytrp, \
                 tc.tile_pool(name="sums", bufs=2) as sumsp, \
                 tc.tile_pool(name="bc", bufs=2) as bcp, \
                 tc.tile_pool(name="rcp", bufs=2) as rcpp, \
                 tc.tile_pool(name="outsb", bufs=3) as outp, \
                 tc.tile_pool(name="st", bufs=3, space="PSUM") as stp, \
                 tc.tile_pool(name="pv", bufs=1, space="PSUM") as pvp:

                tri_b = bass.AP(tri.tensor, tri.offset,
                                [tri.ap[0], [0, 2], tri.ap[1]])

                def attn_fb(qb, fb, yt, inject):
                    """One (query-block, feature-block) attention stream."""
                    nkk = 4 * qb + 4
                    pv = [pvp.tile([65, QB], F32, tag=f"pv{h2}",
                                   name=f"pv{h2}")
                          for h2 in range(2)]
                    pts = {}

                    def issue_S(k):
                        dl = k - 4 * qb
                        j0 = 128 * dl if dl >= 0 else 0
                        st = stp.tile([128, 2, QB], F32, tag="st", name="st")
                        for h2 in range(2):
                            p0, p1 = 64 * h2, 64 * h2 + 64
                            nc.tensor.matmul(
                                st[:, h2, j0:QB],
                                kt_sb[p0:p1, fb, k * 128:(k + 1) * 128],
                                qt_sb[p0:p1, fb, qb * QB + j0:(qb + 1) * QB],
                                start=True, stop=True,
                                skip_group_check=True)
                        pt = ptp.tile([128, 2, QB], F16, tag="pt", name="pt")
                        nc.scalar.activation(
                            pt[:, :, j0:QB], st[:, :, j0:QB], AF.Exp)
                        if dl >= 0:
                            # zero the causally-dead triangle of the diagonal
                            # band (both heads in one strided op; the 0-step
                            # middle dim re-reads the same mask tile)
                            band = pt[:, :, j0:j0 + 128]
                            nc.vector.tensor_mul(band, band, tri_b)
                        pts[k] = (pt, j0)

                    def issue_PV(k):
                        pt, j0 = pts.pop(k)
                        for h2 in range(2):
                            h = 2 * fb + h2
                            nc.tensor.matmul(
                                pv[h2][:, j0:QB], v_sb[:, k, h, :],
                                pt[:, h2, j0:QB],
                                start=(k == 0), stop=(k == nkk - 1),
                                skip_group_check=True)

                    LA = 3
                    for k in range(min(LA, nkk)):
                        issue_S(k)
                    for k in range(nkk):
                        issue_PV(k)
                        if k + LA < nkk:
                            issue_S(k + LA)
                        if inject and k >= 1:
                            inject.pop(0)()
                    while inject:
                        inject.pop(0)()

                    # epilogue fast part: one f32 copy frees each pv bank,
                    # then a small f16 copy of the denominator row feeds the
                    # deferred broadcast matmul
                    ytr = ytrp.tile([65, 2, QB], F32, tag="ytr", name="ytr")
                    sums = sumsp.tile([1, 2, QB], F16, tag="sums",
                                      name="sums")
                    rcp = rcpp.tile([64, 2, QB], F32, tag="rcp", name="rcp")
                    for h2 in range(2):
                        nc.vector.tensor_copy(ytr[:, h2, :], pv[h2])
                        nc.vector.tensor_copy(sums[:, h2, :],
                                              ytr[D:D + 1, h2, :])

                    def finish():
                        # deferred: runs injected into the next stream, so
                        # the broadcast matmul never stalls the PE at a
                        # stream boundary
                        for h2 in range(2):
                            bc = stp.tile([64, QB], F32, tag="st", name="bc")
                            nc.tensor.matmul(bc, ones_col, sums[:, h2, :],
                                             start=True, stop=True,
                                             skip_group_check=True)
                            nc.vector.reciprocal_approx_fast(
                                out=rcp[:, h2, :], in_=bc)
                            nc.vector.tensor_mul(
                                yt[64 * h2:64 * h2 + 64, fb, :],
                                ytr[0:D, h2, :], rcp[:, h2, :])
                    return finish

                def make_prj(qb, yt, osbs):
                    """Closures for qb's 8 output-projection chains."""
                    def chain(j):
                        def run():
                            tt, ofc = j // 2, j % 2
                            if ofc == 0:
                                osbs.append(outp.tile([128, C], F32,
                                                      tag="osb", name="osb"))
                            osb = osbs[-1]
                            prjt = stp.tile([128, QB], F32, tag="st",
                                            name="prjt")
                            for cc in range(NFB):
                                nc.tensor.matmul(
                                    prjt,
                                    yt[:, cc, tt * 128:(tt + 1) * 128],
                                    wp_sb[:, cc, ofc * 512:(ofc + 1) * 512],
                                    start=(cc == 0), stop=(cc == NFB - 1),
                                    skip_group_check=True)
                            nc.vector.tensor_copy(
                                osb[:, ofc * 512:(ofc + 1) * 512], prjt)
                            if ofc == 1:
                                r0 = qb * QB + tt * 128
                                nc.sync.dma_start(out_d[r0:r0 + 128, :], osb)
                        return run
                    return [chain(j) for j in range(8)]

                yts = {}
                osbs = []
                pending_finish = None
                for qb in range(NQB):
                    yts[qb] = ytp.tile([128, NFB, QB], F16, tag="yt",
                                       name="yt")
                    for fb in range(NFB):
                        inject = []
                        if pending_finish is not None:
                            inject.append(pending_finish)
                        if fb == 0 and qb > 0:
                            inject.extend(make_prj(qb - 1, yts[qb - 1],
                                                   osbs))
                        pending_finish = attn_fb(qb, fb, yts[qb], inject)
                # drain: last stream's softmax finish + its projection
                pending_finish()
                for run in make_prj(NQB - 1, yts[NQB - 1], osbs):
                    run()

    nc.compile()
    return nc


def _host_inputs(x, Wk, Wq, Wv, Wp):
    """Build the 8 per-core input maps (host-side slicing/transposes)."""
    p = np.arange(128)[:, None]
    jj = np.arange(128)[None, :]
    tri_np = np.where(jj < p, 0.0, 1.0).astype(np.float16)

    in_maps = []
    for c in range(N_CORES):
        b, hg = c // 2, c % 2
        fs = slice(hg * FPC, (hg + 1) * FPC)
        in_maps.append({
            "xT": np.ascontiguousarray(x[b].T).astype(np.float16),
            "wqT": np.ascontiguousarray((Wq[fs, :] * 0.125).T).astype(np.float16),
            "wkT": np.ascontiguousarray(Wk[fs, :].T).astype(np.float16),
            "wvT": np.ascontiguousarray(Wv[fs, :].T).astype(np.float16),
            "wpT": np.ascontiguousarray(Wp[:, fs].T).astype(np.float16),
            "tri": tri_np,
        })
    return in_maps


def kernel(x, Wk, Wq, Wv, Wp, bp, _trace=False):
    x = np.asarray(x, dtype=np.float32)
    Wk = np.asarray(Wk, dtype=np.float32)
    Wq = np.asarray(Wq, dtype=np.float32)
    Wv = np.asarray(Wv, dtype=np.float32)
    Wp = np.asarray(Wp, dtype=np.float32)
    bp = np.asarray(bp, dtype=np.float32)

    if "nc" not in _cached:
        _cached["nc"] = _build_program()
    nc = _cached["nc"]

    in_maps = _host_inputs(x, Wk, Wq, Wv, Wp)
    res = run_bass_kernel_spmd(nc, in_maps, core_ids=list(range(N_CORES)),
                               trace=_trace)
    _cached["last_result"] = res

    out = np.empty((B, T, C), dtype=np.float32)
    for b in range(B):
        out[b] = (res.results[2 * b]["out"].astype(np.float32)
                  + res.results[2 * b + 1]["out"]
                  + bp[None, :])
    return out
